# revision 1
# baseline (speedup 1.0000x reference)
"""Trainium2 Bass kernel for nn_ConcatenatedIrrepsTensorProduct.

Strategy: pure data-parallel over the edge dimension E=200000 across 8
NeuronCores (25000 edges each, zero-padded to 25088 = 49 tiles x 512 edges).
Small weight matrices are folded/permuted on the host and replicated to all
cores.

Per 512-edge tile (4 blocks of 128 edges), everything runs feature-major:
  1. T-stage: PE matmuls  X_blk.T @ [I | I]  (N=256 keeps float32r matmuls
     at full rate; the duplicate half is discarded by the strided evacuate).
  2. MLP: 3 matmuls + 2 Silu activations (radial weight generator); the
     third-layer weights are column-duplicated on host so the outputs land
     exactly aligned with the tensor-product row stacks (WTA/WTB/WTBx).
  3. G-stage: stacked main-TP matmuls (host-built block lhsT with the
     interleaved (m,k) feature order baked in).
  4. Elementwise: multiply by y0/y1k broadcast rows (built by K=4 selector
     matmuls from the transposed x2 rows) and by the MLP weights.
  5. F-stage: folded (Wl@Wf) matmuls, then PE transpose back to edge-major.
"""

import numpy as np

import concourse.bacc as bacc
import concourse.bass as bass
import concourse.mybir as mybir
import concourse.tile as tile
from concourse.bass_utils import run_bass_kernel_spmd

# ----------------------------------------------------------------------------
# problem constants (hardcoded; kernel.py must be self-contained)
E = 200000
NCORES = 8
EC = E // NCORES            # 25000 edges per core
TILE = 512                  # edges per tile
NBLK = 4                    # 128-edge blocks per tile
NT = (EC + TILE - 1) // TILE  # 49
ECP = NT * TILE             # 25088 padded edges per core

MUL = 32
U = 64
SCALAR_DIM = 64
HID = 64
PW = 0.125
INV_S3 = 1.0 / np.sqrt(3.0)

F32 = mybir.dt.float32
USE_F32R = True             # float32r = full-rate PE matmuls (reduced precision)
MMDT = mybir.dt.float32r if USE_F32R else F32

_CACHE = {}


def _silu_cst() -> float:
    z = np.linspace(-12.0, 12.0, 200001)
    phi = np.exp(-0.5 * z**2) / np.sqrt(2.0 * np.pi)
    s = z / (1.0 + np.exp(-z))
    return float(1.0 / np.sqrt(np.trapezoid(s**2 * phi, z)))


# ----------------------------------------------------------------------------
# host-side constant folding


def build_consts(w0, w1, w2, w3, Wl0, Wl1, Wm1, Wm2, Wm3, Wf0, Wf1):
    """Fold all weights into the matrices the device kernel consumes."""
    f8 = 1.0 / np.sqrt(np.float64(U))          # 1/8
    fm = 1.0 / np.sqrt(np.float64(MUL))        # 1/sqrt(32)
    C = _silu_cst()

    w0p = (PW * w0).astype(np.float64)
    w1p = (PW * INV_S3 * w1).astype(np.float64)
    w2p = (PW * w2).astype(np.float64)
    w3p = (PW * w3).astype(np.float64)
    Wc0 = (Wl0.astype(np.float64) @ Wf0.astype(np.float64)) * (f8 * fm)  # [64,32]
    Wc1 = (Wl1.astype(np.float64) @ Wf1.astype(np.float64)) * (f8 * fm)  # [64,32]

    # --- G-stage lhsT matrices -------------------------------------------
    # GA rows: [mid0_raw(32); t3k = w3'.s1 for k=0,1,2 (3x32)]
    LA_a = np.zeros((128, 128))
    LA_b = np.zeros((128, 128))
    for r in range(32):                       # mid0, w=r
        for f in range(32):                   # s0 features (u = f / 32+f)
            LA_a[f, r] = w0p[f, r]
            LA_b[f, r] = w0p[32 + f, r]
    for k in range(3):
        for v in range(32):
            r = 32 + 32 * k + v               # t3k, w=v
            for m in range(32):
                f = 32 + 3 * m + k            # s1[m,k] feature column
                LA_a[f, r] = w3p[m, v]
                LA_b[f, r] = w3p[32 + m, v]

    # GB rows: [t10(32); t11(32); t12(32); mid2_raw#k0(32)]
    LB_a = np.zeros((128, 128))
    LB_b = np.zeros((128, 128))
    for k in range(3):
        for v in range(32):
            r = 32 * k + v                    # t1k, w=v
            for m in range(32):
                f = 32 + 3 * m + k
                LB_a[f, r] = w1p[m, v]
                LB_b[f, r] = w1p[32 + m, v]
    for v in range(32):
        r = 96 + v                            # mid2_raw (copy for k=0)
        for f in range(32):
            LB_a[f, r] = w2p[f, v]
            LB_b[f, r] = w2p[32 + f, v]

    # GBx rows: [mid2_raw#k1(32); mid2_raw#k2(32)]
    LBx_a = np.zeros((128, 64))
    LBx_b = np.zeros((128, 64))
    for j in range(2):
        for v in range(32):
            r = 32 * j + v
            for f in range(32):
                LBx_a[f, r] = w2p[f, v]
                LBx_b[f, r] = w2p[32 + f, v]

    # --- broadcast selectors (K=4 over transposed x2 rows [y0,y10,y11,y12])
    SelA = np.zeros((4, 128))
    SelA[0, :] = 1.0                          # whole GA stack scales by y0
    Sel1 = np.zeros((4, 128))
    for r in range(96):
        Sel1[1 + r // 32, r] = 1.0            # t1k rows get y1k
    for r in range(96, 128):
        Sel1[1, r] = 1.0                      # mid2#k0 rows get y10
    Sel2 = np.zeros((4, 64))
    for r in range(32):
        Sel2[2, r] = 1.0                      # mid2#k1 -> y11
    for r in range(32, 64):
        Sel2[3, r] = 1.0                      # mid2#k2 -> y12

    # --- MLP weights ------------------------------------------------------
    Wm1_8 = Wm1.astype(np.float64) / np.sqrt(np.float64(SCALAR_DIM))
    Wm2s = C * Wm2.astype(np.float64) / np.sqrt(np.float64(HID))
    Wm3s = C * Wm3.astype(np.float64) / np.sqrt(np.float64(HID))  # [64,128]

    # MLP3 column-duplicated variants aligned with the row stacks
    cmA = np.zeros(128, dtype=np.int64)
    cmA[:32] = np.arange(32)                  # mid0 rows -> wt[r]
    for k in range(3):
        for v in range(32):
            cmA[32 + 32 * k + v] = 96 + v     # t3k rows -> wt[96+v]
    cmB = np.zeros(128, dtype=np.int64)
    for k in range(3):
        for v in range(32):
            cmB[32 * k + v] = 32 + v          # t1k rows -> wt[32+v] (mid1 slot)
    cmB[96:] = 64 + np.arange(32)             # mid2#k0 rows -> wt[64+v]
    cmBx = np.concatenate([64 + np.arange(32), 64 + np.arange(32)])
    Wm3A = Wm3s[:, cmA]
    Wm3B = Wm3s[:, cmB]
    Wm3Bx = Wm3s[:, cmBx]                     # [64, 64]

    # --- F-stage lhsT (folded Wl@Wf + output interleave) ------------------
    WfA = np.zeros((128, 128))
    for r in range(32):                       # m0[r] (mid0 part)
        WfA[r, :32] = Wc0[r, :]
    for k in range(3):
        for v in range(32):
            r = 32 + 32 * k + v               # m1k[32+v] (mid3k part)
            for w in range(32):
                WfA[r, 32 + 3 * w + k] = Wc1[32 + v, w]
    WfB = np.zeros((128, 128))
    for k in range(3):
        for v in range(32):
            r = 32 * k + v                    # mid1 contribution
            WfB[r, :32] = Wc0[32 + v, :]
    for v in range(32):
        r = 96 + v                            # m10[v] (mid2_0 part)
        for w in range(32):
            WfB[r, 32 + 3 * w + 0] = Wc1[v, w]
    WfBx = np.zeros((64, 128))
    for j, k in ((0, 1), (1, 2)):
        for v in range(32):
            r = 32 * j + v
            for w in range(32):
                WfBx[r, 32 + 3 * w + k] = Wc1[v, w]

    f32 = np.float32
    eye = np.eye(128, dtype=f32)
    return {
        "Wm2s_hi": Wm2s.astype(f32), "Wm3A_hi": Wm3A.astype(f32),
        "Wm3B_hi": Wm3B.astype(f32), "Wm3Bx_hi": Wm3Bx.astype(f32),
        "II": np.concatenate([eye, eye], axis=1),   # [128, 256]
        "LA_a": LA_a.astype(f32), "LA_b": LA_b.astype(f32),
        "LB_a": LB_a.astype(f32), "LB_b": LB_b.astype(f32),
        "LBx_a": LBx_a.astype(f32), "LBx_b": LBx_b.astype(f32),
        "SelA": SelA.astype(f32), "Sel1": Sel1.astype(f32),
        "Sel2": Sel2.astype(f32),
        "Wm1_8": Wm1_8.astype(f32), "Wm2s": Wm2s.astype(f32),
        "Wm3A": Wm3A.astype(f32), "Wm3B": Wm3B.astype(f32),
        "Wm3Bx": Wm3Bx.astype(f32),
        "WfA": WfA.astype(f32), "WfB": WfB.astype(f32),
        "WfBx": WfBx.astype(f32),
    }


# const blob layout: every const packed column-wise into one [128, CB_COLS]
# array (one DMA, one sem lane).  The 4-row selectors overlay rows 64:68 of
# the Wm3* column ranges (those only occupy rows 0:64).
CB_LAYOUT = {}


def _mk_layout():
    off = 0
    for n, p, w in (
        ("II", 128, 256),
        ("LA_a", 128, 128), ("LA_b", 128, 128),
        ("LB_a", 128, 128), ("LB_b", 128, 128),
        ("LBx_a", 128, 64), ("LBx_b", 128, 64),
        ("Wm1_8", 64, 64), ("Wm2s", 64, 64),
        ("Wm3A", 64, 128), ("Wm3B", 64, 128), ("Wm3Bx", 64, 64),
        ("WfA", 128, 128), ("WfB", 128, 128), ("WfBx", 64, 128),
    ):
        CB_LAYOUT[n] = (0, p, off, w)
        off += w
    # base-64 copies for the odd tile of each MLP pair
    for n, w in (("Wm2s_hi", 64), ("Wm3A_hi", 128), ("Wm3B_hi", 128),
                 ("Wm3Bx_hi", 64)):
        CB_LAYOUT[n] = (64, 64, off, w)
        off += w
    # overlays (rows 64:68)
    CB_LAYOUT["SelA"] = (64, 4, CB_LAYOUT["Wm3A"][2], 128)
    CB_LAYOUT["Sel1"] = (64, 4, CB_LAYOUT["Wm3B"][2], 128)
    CB_LAYOUT["Sel2"] = (64, 4, CB_LAYOUT["Wm3Bx"][2], 64)
    return off


CB_COLS = _mk_layout()


def pack_consts(consts):
    cb = np.zeros((128, CB_COLS), dtype=np.float32)
    for n, (r0, p, c0, w) in CB_LAYOUT.items():
        cb[r0:r0 + p, c0:c0 + w] = consts[n]
    return cb


# ----------------------------------------------------------------------------
# device kernel


def build_nc():
    nc = bacc.Bacc("TRN2", target_bir_lowering=False)

    x1a_d = nc.declare_dram_parameter("x1a", [ECP, 128], MMDT, isOutput=False)
    x1b_d = nc.declare_dram_parameter("x1b", [ECP, 128], MMDT, isOutput=False)
    scx2_d = nc.declare_dram_parameter("scx2", [ECP, 68], MMDT, isOutput=False)
    cb_d = nc.declare_dram_parameter("CB", [128, CB_COLS], MMDT, isOutput=False)
    out_d = nc.declare_dram_parameter("out", [ECP, 128], F32, isOutput=True)

    SILU = mybir.ActivationFunctionType.Silu

    with tile.TileContext(nc) as tc:
        with (
            tc.tile_pool(name="consts", bufs=1) as cpool,
            tc.tile_pool(name="xin", bufs=3) as xpool,
            tc.tile_pool(name="vsb", bufs=3) as vpool,
            tc.tile_pool(name="mid", bufs=3) as mpool,
            tc.tile_pool(name="outp", bufs=3) as opool,
            tc.tile_pool(name="ps", bufs=1, space="PSUM") as ps,
        ):
            # ---- load constants once (single DMA) -----------------------
            cb = cpool.tile([128, CB_COLS], MMDT, tag="cb", name="cb")
            nc.sync.dma_start(cb[:], cb_d[:])
            csb = {
                n: cb[r0:r0 + p, c0:c0 + w]
                for n, (r0, p, c0, w) in CB_LAYOUT.items()
            }

            NPAIR = (NT + 1) // 2
            for pr in range(NPAIR):
                e0 = pr * 2 * TILE
                n_t = min(2 * TILE, ECP - e0)
                n_tiles = n_t // TILE

                # ---- input DMAs: one per tensor per pair ----------------
                xa2 = xpool.tile([128, 2 * NBLK, 128], MMDT, tag="xa2")
                nc.sync.dma_start(
                    xa2[:, : n_t // 128, :],
                    x1a_d[e0:e0 + n_t, :].rearrange("(b p) f -> p b f", p=128))
                xb2 = xpool.tile([128, 2 * NBLK, 128], MMDT, tag="xb2")
                nc.sync.dma_start(
                    xb2[:, : n_t // 128, :],
                    x1b_d[e0:e0 + n_t, :].rearrange("(b p) f -> p b f", p=128))
                scx22 = xpool.tile([128, 2 * NBLK, 68], MMDT, tag="scx22")
                nc.sync.dma_start(
                    scx22[:, : n_t // 128, :],
                    scx2_d[e0:e0 + n_t, :].rearrange("(b p) f -> p b f", p=128))

                # ---- T-stage for both tiles of the pair ------------------
                # One psum bank per input stream: blocks overlap-chained so
                # block b's real half lands at cols [128b:128b+128]; the
                # [I|I] junk half of each matmul is overwritten by the next
                # block (the last block uses N=128, no junk).
                vas, vbs, scTs = [], [], []
                for h in range(n_tiles):
                    xa = xa2[:, h * NBLK:(h + 1) * NBLK, :]
                    xb = xb2[:, h * NBLK:(h + 1) * NBLK, :]
                    sx = scx22[:, h * NBLK:(h + 1) * NBLK, :]
                    pva = ps.tile([128, 512], F32, tag="TA", name="pva")
                    pvb = ps.tile([128, 512], F32, tag="TB", name="pvb")
                    psc = ps.tile([128, 512], F32, tag="TC", name="psc")
                    for b in range(NBLK):
                        if b < NBLK - 1:
                            rhs, o = csb["II"], slice(128 * b, 128 * b + 256)
                        else:
                            rhs, o = csb["II"][:, 0:128], slice(128 * b, 512)
                        nc.tensor.matmul(pva[:, o], xa[:, b, :], rhs,
                                         start=True, stop=True,
                                         skip_group_check=True)
                        nc.tensor.matmul(pvb[:, o], xb[:, b, :], rhs,
                                         start=True, stop=True,
                                         skip_group_check=True)
                        nc.tensor.matmul(psc[0:68, o], sx[:, b, :], rhs,
                                         start=True, stop=True,
                                         skip_group_check=True)
                    va = vpool.tile([128, NBLK, 128], MMDT, tag=f"va{h}",
                                    name=f"va{h}")
                    vb = vpool.tile([128, NBLK, 128], MMDT, tag=f"vb{h}",
                                    name=f"vb{h}")
                    scT = vpool.tile([68, NBLK, 128], MMDT, tag=f"scT{h}",
                                     name=f"scT{h}")
                    nc.scalar.copy(va[:], pva[:].rearrange("q (b f) -> q b f", f=128))
                    nc.scalar.copy(vb[:], pvb[:].rearrange("q (b f) -> q b f", f=128))
                    nc.vector.tensor_copy(
                        scT[:], psc[0:68, :].rearrange("q (b f) -> q b f", f=128))
                    vas.append(va); vbs.append(vb); scTs.append(scT)

                # ---- per-tile tail: MLP, WT, B, G, scaling, F, out ------
                for h in range(n_tiles):
                    t_i = 2 * pr + h
                    te0 = t_i * TILE
                    va, vb, scT = vas[h], vbs[h], scTs[h]

                    scT_f = scT.rearrange("q b f -> q (b f)")
                    p1 = ps.tile([64, 512], F32, tag="TF", name="p1")
                    nc.tensor.matmul(p1[:], csb["Wm1_8"], scT_f[0:64, :],
                                     start=True, stop=True)
                    a1 = mpool.tile([64, 512], MMDT, tag="a1")
                    nc.scalar.activation(a1[:], p1[:], SILU)
                    p2 = ps.tile([64, 512], F32, tag="TF", name="p2")
                    nc.tensor.matmul(p2[:], csb["Wm2s"], a1[:],
                                     start=True, stop=True)
                    a2h = mpool.tile([64, 512], MMDT, tag="a2")
                    nc.scalar.activation(a2h[:], p2[:], SILU)

                    pWTA = ps.tile([128, 512], F32, tag="TE", name="pWTA")
                    nc.tensor.matmul(pWTA[:], csb["Wm3A"], a2h[:],
                                     start=True, stop=True)
                    pWTB = ps.tile([128, 512], F32, tag="TG", name="pWTB")
                    nc.tensor.matmul(pWTB[:], csb["Wm3B"], a2h[:],
                                     start=True, stop=True)
                    pWTBx = ps.tile([64, 512], F32, tag="TF", name="pWTBx")
                    nc.tensor.matmul(pWTBx[:], csb["Wm3Bx"], a2h[:],
                                     start=True, stop=True)

                    y_rows = scT[64:68, :, :].rearrange("q b f -> q (b f)")
                    pBA = ps.tile([128, 512], F32, tag="TH", name="pBA")
                    nc.tensor.matmul(pBA[:], csb["SelA"], y_rows,
                                     start=True, stop=True)
                    pB1 = ps.tile([128, 512], F32, tag="TD", name="pB1")
                    nc.tensor.matmul(pB1[:], csb["Sel1"], y_rows,
                                     start=True, stop=True)
                    pB2 = ps.tile([64, 512], F32, tag="TC", name="pB2")
                    nc.tensor.matmul(pB2[:], csb["Sel2"], y_rows,
                                     start=True, stop=True)
                    BA = mpool.tile([128, 512], F32, tag="BA")
                    nc.scalar.copy(BA[:], pBA[:])
                    B1 = mpool.tile([128, 512], F32, tag="B1")
                    nc.vector.tensor_copy(B1[:], pB1[:])
                    B2 = mpool.tile([64, 512], F32, tag="B2")
                    nc.scalar.copy(B2[:], pB2[:])

                    pGA = ps.tile([128, 512], F32, tag="TH", name="pGA")
                    pGB = ps.tile([128, 512], F32, tag="TD", name="pGB")
                    pGBx = ps.tile([64, 512], F32, tag="TC", name="pGBx")
                    for si, (v, la, lb, lx) in enumerate(
                        ((va, "LA_a", "LB_a", "LBx_a"),
                         (vb, "LA_b", "LB_b", "LBx_b"))
                    ):
                        st = si == 0
                        sp = si == 1
                        nc.tensor.matmul(pGA[:], csb[la], v[:], start=st, stop=sp)
                        nc.tensor.matmul(pGB[:], csb[lb], v[:], start=st, stop=sp)
                        nc.tensor.matmul(pGBx[:], csb[lx], v[:], start=st, stop=sp)

                    YA = mpool.tile([128, 512], F32, tag="YA")
                    nc.vector.tensor_mul(YA[:], pGA[:], BA[:])
                    Y1 = mpool.tile([128, 512], F32, tag="Y1")
                    nc.vector.tensor_mul(Y1[:], pGB[:], B1[:])
                    Y2 = mpool.tile([64, 512], F32, tag="Y2")
                    nc.vector.tensor_mul(Y2[:], pGBx[:], B2[:])
                    MsA = mpool.tile([128, 512], MMDT, tag="MsA")
                    nc.vector.tensor_mul(MsA[:], YA[:], pWTA[:])
                    MsB = mpool.tile([128, 512], MMDT, tag="MsB")
                    nc.vector.tensor_mul(MsB[:], Y1[:], pWTB[:])
                    MsBx = mpool.tile([64, 512], MMDT, tag="MsBx")
                    nc.vector.tensor_mul(MsBx[:], Y2[:], pWTBx[:])

                    pOF = ps.tile([128, 512], F32, tag="TE", name="pOF")
                    nc.tensor.matmul(pOF[:], csb["WfA"], MsA[:],
                                     start=True, stop=False)
                    nc.tensor.matmul(pOF[:], csb["WfB"], MsB[:],
                                     start=False, stop=False)
                    nc.tensor.matmul(pOF[:], csb["WfBx"], MsBx[:],
                                     start=False, stop=True)
                    OF = opool.tile([128, NBLK, 128], MMDT, tag="OF")
                    nc.scalar.copy(OF[:], pOF[:].rearrange("q (b f) -> q b f", f=128))

                    # transpose back, overlap-chained into two banks
                    pOT = [ps.tile([128, 512], F32, tag="TH", name="pOT0"),
                           ps.tile([128, 512], F32, tag="TD", name="pOT1")]
                    for b in range(NBLK):
                        q = b % 2
                        if b < 2:
                            rhs, o = csb["II"], slice(0, 256)
                        else:
                            rhs, o = csb["II"][:, 0:128], slice(128, 256)
                        nc.tensor.matmul(
                            pOT[q][:, o], OF[:, b, :], rhs,
                            start=True, stop=True, skip_group_check=True)
                    out_sb = opool.tile([128, NBLK, 128], F32, tag="out_sb")
                    for q in range(2):
                        nc.scalar.copy(
                            out_sb[:, q::2, :],
                            pOT[q][:, 0:256].rearrange("e (c f) -> e c f", f=128))
                    nc.sync.dma_start(
                        out_d[te0:te0 + TILE, :].rearrange("(b p) f -> p b f", p=128),
                        out_sb[:])

    nc.finalize()
    return nc


def _get_nc():
    if "nc" not in _CACHE:
        _CACHE["nc"] = build_nc()
    return _CACHE["nc"]


# ----------------------------------------------------------------------------
# host entry point


def _pad(a):
    return np.pad(a, ((0, ECP - EC), (0, 0)))


def run(inputs, trace=False):
    inputs = {k: np.asarray(v) for k, v in inputs.items()}
    consts = build_consts(
        inputs["w0"], inputs["w1"], inputs["w2"], inputs["w3"],
        inputs["Wl0"], inputs["Wl1"],
        inputs["Wm1"], inputs["Wm2"], inputs["Wm3"],
        inputs["Wf0"], inputs["Wf1"],
    )
    cb = pack_consts(consts)
    nc = _get_nc()
    in_maps = []
    for c in range(NCORES):
        s = slice(c * EC, (c + 1) * EC)
        m = {
            "x1a": _pad(np.ascontiguousarray(inputs["x1a"][s])),
            "x1b": _pad(np.ascontiguousarray(inputs["x1b"][s])),
            "scx2": _pad(np.concatenate(
                [inputs["scalars"][s], inputs["x2"][s]], axis=1)),
            "CB": cb,
        }
        in_maps.append(m)
    res = run_bass_kernel_spmd(nc, in_maps, list(range(NCORES)), trace=trace)
    out = np.concatenate([res.results[c]["out"][:EC] for c in range(NCORES)], axis=0)
    return out, res


def kernel(**inputs) -> np.ndarray:
    out, _ = run(inputs, trace=False)
    return out



# revision 22
# speedup vs baseline: 169.4089x; 169.4089x over previous
"""Trainium2 Bass kernel for nn_ConcatenatedIrrepsTensorProduct.

Strategy: pure data-parallel over the edge dimension E=200000 across 8
NeuronCores (25000 edges each, zero-padded to 25088 = 49 tiles x 512 edges).
Small weight matrices are folded/permuted on the host and replicated.

All per-edge tensors are shipped FEATURE-MAJOR (transposed on the host), so
the device never transposes: every stage is a [K<=128, 512]-rhs matmul over
feature partitions, with edges on the free axis.

Per 512-edge tile:
  1. MLP: 3 matmuls + 2 Silu activations (radial weight generator); the
     third-layer weights are column-duplicated on host so outputs land
     aligned with the tensor-product row stacks (WTA/WTB/WTBx).
  2. Sel: K=4 selector matmuls build y0/y1k broadcast rows from the
     transposed x2 rows.
  3. G: stacked main-TP matmuls (host-built block lhsT with the
     interleaved (m,k) feature order baked in), rhs = raw x1aT/x1bT tiles.
  4. Elementwise: multiply by broadcast rows and MLP weights (DVE).
  5. F: folded (Wl@Wf) matmuls -> feature-major output tile, DMA'd out.
The host transposes the output back to edge-major.
"""

import numpy as np

import concourse.bacc as bacc
import concourse.bass as bass
import concourse.mybir as mybir
import concourse.tile as tile
from concourse.bass_utils import run_bass_kernel_spmd

# ----------------------------------------------------------------------------
# problem constants (hardcoded; kernel.py must be self-contained)
E = 200000
NCORES = 8
EC = E // NCORES            # 25000 edges per core
TILE = 512                  # edges per tile
NT = (EC + TILE - 1) // TILE  # 49
ECP = NT * TILE             # 25088 padded edges per core

MUL = 32
U = 64
SCALAR_DIM = 64
HID = 64
PW = 0.125
INV_S3 = 1.0 / np.sqrt(3.0)

F32 = mybir.dt.float32
MMDT = mybir.dt.bfloat16    # bf16 matmul operands: FWL weight loads + half DMA
NPBF16 = mybir.dt.np(MMDT)

_CACHE = {}


def _silu_cst() -> float:
    z = np.linspace(-12.0, 12.0, 200001)
    phi = np.exp(-0.5 * z**2) / np.sqrt(2.0 * np.pi)
    s = z / (1.0 + np.exp(-z))
    trapz = np.trapz if hasattr(np, "trapz") else np.trapezoid
    return float(1.0 / np.sqrt(trapz(s**2 * phi, z)))


# ----------------------------------------------------------------------------
# host-side constant folding


def build_consts(w0, w1, w2, w3, Wl0, Wl1, Wm1, Wm2, Wm3, Wf0, Wf1):
    """Fold all weights into the matrices the device kernel consumes."""
    f8 = 1.0 / np.sqrt(np.float64(U))          # 1/8
    fm = 1.0 / np.sqrt(np.float64(MUL))        # 1/sqrt(32)
    C = _silu_cst()

    w0p = (PW * w0).astype(np.float64)
    w1p = (PW * INV_S3 * w1).astype(np.float64)
    w2p = (PW * w2).astype(np.float64)
    w3p = (PW * w3).astype(np.float64)
    Wc0 = (Wl0.astype(np.float64) @ Wf0.astype(np.float64)) * (f8 * fm)  # [64,32]
    Wc1 = (Wl1.astype(np.float64) @ Wf1.astype(np.float64)) * (f8 * fm)  # [64,32]

    # --- G-stage lhsT matrices -------------------------------------------
    # GA rows: [mid0_raw(32); t3k = w3'.s1 for k=0,1,2 (3x32)]
    LA_a = np.zeros((128, 128))
    LA_b = np.zeros((128, 128))
    for r in range(32):                       # mid0, w=r
        for f in range(32):                   # s0 features (u = f / 32+f)
            LA_a[f, r] = w0p[f, r]
            LA_b[f, r] = w0p[32 + f, r]
    for k in range(3):
        for v in range(32):
            r = 32 + 32 * k + v               # t3k, w=v
            for m in range(32):
                f = 32 + 3 * m + k            # s1[m,k] feature column
                LA_a[f, r] = w3p[m, v]
                LA_b[f, r] = w3p[32 + m, v]

    # GB rows: [t10(32); t11(32); t12(32); mid2_raw#k0(32)]
    LB_a = np.zeros((128, 128))
    LB_b = np.zeros((128, 128))
    for k in range(3):
        for v in range(32):
            r = 32 * k + v                    # t1k, w=v
            for m in range(32):
                f = 32 + 3 * m + k
                LB_a[f, r] = w1p[m, v]
                LB_b[f, r] = w1p[32 + m, v]
    for v in range(32):
        r = 96 + v                            # mid2_raw (copy for k=0)
        for f in range(32):
            LB_a[f, r] = w2p[f, v]
            LB_b[f, r] = w2p[32 + f, v]

    # GBx rows: [mid2_raw#k1(32); mid2_raw#k2(32)]
    LBx_a = np.zeros((128, 64))
    LBx_b = np.zeros((128, 64))
    for j in range(2):
        for v in range(32):
            r = 32 * j + v
            for f in range(32):
                LBx_a[f, r] = w2p[f, v]
                LBx_b[f, r] = w2p[32 + f, v]

    # --- broadcast selectors (K=4 over transposed x2 rows [y0,y10,y11,y12])
    SelA = np.zeros((4, 128))
    SelA[0, :] = 1.0                          # whole GA stack scales by y0
    Sel1 = np.zeros((4, 128))
    for r in range(96):
        Sel1[1 + r // 32, r] = 1.0            # t1k rows get y1k
    for r in range(96, 128):
        Sel1[1, r] = 1.0                      # mid2#k0 rows get y10
    Sel2 = np.zeros((4, 64))
    for r in range(32):
        Sel2[2, r] = 1.0                      # mid2#k1 -> y11
    for r in range(32, 64):
        Sel2[3, r] = 1.0                      # mid2#k2 -> y12

    # --- MLP weights ------------------------------------------------------
    Wm1_8 = Wm1.astype(np.float64) / np.sqrt(np.float64(SCALAR_DIM))
    Wm2s = C * Wm2.astype(np.float64) / np.sqrt(np.float64(HID))
    Wm3s = C * Wm3.astype(np.float64) / np.sqrt(np.float64(HID))  # [64,128]

    # MLP3 column-duplicated variants aligned with the row stacks
    cmA = np.zeros(128, dtype=np.int64)
    cmA[:32] = np.arange(32)                  # mid0 rows -> wt[r]
    for k in range(3):
        for v in range(32):
            cmA[32 + 32 * k + v] = 96 + v     # t3k rows -> wt[96+v]
    cmB = np.zeros(128, dtype=np.int64)
    for k in range(3):
        for v in range(32):
            cmB[32 * k + v] = 32 + v          # t1k rows -> wt[32+v] (mid1 slot)
    cmB[96:] = 64 + np.arange(32)             # mid2#k0 rows -> wt[64+v]
    cmBx = np.concatenate([64 + np.arange(32), 64 + np.arange(32)])
    Wm3A = Wm3s[:, cmA]
    Wm3B = Wm3s[:, cmB]
    Wm3Bx = Wm3s[:, cmBx]                     # [64, 64]

    # --- F-stage lhsT (folded Wl@Wf + output interleave) ------------------
    WfA = np.zeros((128, 128))
    for r in range(32):                       # m0[r] (mid0 part)
        WfA[r, :32] = Wc0[r, :]
    for k in range(3):
        for v in range(32):
            r = 32 + 32 * k + v               # m1k[32+v] (mid3k part)
            for w in range(32):
                WfA[r, 32 + 3 * w + k] = Wc1[32 + v, w]
    WfB = np.zeros((128, 128))
    for k in range(3):
        for v in range(32):
            r = 32 * k + v                    # mid1 contribution
            WfB[r, :32] = Wc0[32 + v, :]
    for v in range(32):
        r = 96 + v                            # m10[v] (mid2_0 part)
        for w in range(32):
            WfB[r, 32 + 3 * w + 0] = Wc1[v, w]
    WfBx = np.zeros((64, 128))
    for j, k in ((0, 1), (1, 2)):
        for v in range(32):
            r = 32 * j + v
            for w in range(32):
                WfBx[r, 32 + 3 * w + k] = Wc1[v, w]

    f32 = np.float32
    return {
        "LA_a": LA_a.astype(f32), "LA_b": LA_b.astype(f32),
        "LB_a": LB_a.astype(f32), "LB_b": LB_b.astype(f32),
        "LBx_a": LBx_a.astype(f32), "LBx_b": LBx_b.astype(f32),
        "SelA": SelA.astype(f32), "Sel1": Sel1.astype(f32),
        "Sel2": Sel2.astype(f32),
        "Wm1_8": Wm1_8.astype(f32), "Wm2s": Wm2s.astype(f32),
        "Wm3A": Wm3A.astype(f32), "Wm3B": Wm3B.astype(f32),
        "Wm3Bx": Wm3Bx.astype(f32),
        "WfA": WfA.astype(f32), "WfB": WfB.astype(f32),
        "WfBx": WfBx.astype(f32),
    }


# const blob layout: every const packed column-wise into one [128, CB_COLS]
# array (one DMA, one sem lane).  The 4-row selectors overlay rows 64:68 of
# the Wm3* column ranges (those only occupy rows 0:64).
CB_LAYOUT = {}


def _mk_layout():
    off = 0
    for n, p, w in (
        ("LA_a", 128, 128), ("LA_b", 128, 128),
        ("LB_a", 128, 128), ("LB_b", 128, 128),
        ("LBx_a", 128, 64), ("LBx_b", 128, 64),
        ("Wm1_8", 64, 64), ("Wm2s", 64, 64),
        ("Wm3A", 64, 128), ("Wm3B", 64, 128), ("Wm3Bx", 64, 64),
        ("WfA", 128, 128), ("WfB", 128, 128), ("WfBx", 64, 128),
    ):
        CB_LAYOUT[n] = (0, p, off, w)
        off += w
    # overlays (rows 64:68)
    CB_LAYOUT["SelA"] = (64, 4, CB_LAYOUT["Wm3A"][2], 128)
    CB_LAYOUT["Sel1"] = (64, 4, CB_LAYOUT["Wm3B"][2], 128)
    CB_LAYOUT["Sel2"] = (64, 4, CB_LAYOUT["Wm3Bx"][2], 64)
    return off


CB_COLS = _mk_layout()


def pack_consts(consts):
    cb = np.zeros((128, CB_COLS), dtype=np.float32)
    for n, (r0, p, c0, w) in CB_LAYOUT.items():
        cb[r0:r0 + p, c0:c0 + w] = consts[n]
    return cb.astype(NPBF16)


# ----------------------------------------------------------------------------
# device kernel


def build_nc():
    nc = bacc.Bacc("TRN2", target_bir_lowering=False)

    xa_d = nc.declare_dram_parameter("x1aT", [128, ECP], MMDT, isOutput=False)
    xb_d = nc.declare_dram_parameter("x1bT", [128, ECP], MMDT, isOutput=False)
    sc_d = nc.declare_dram_parameter("scx2T", [68, ECP], MMDT, isOutput=False)
    cb_d = nc.declare_dram_parameter("CB", [128, CB_COLS], MMDT, isOutput=False)
    out_d = nc.declare_dram_parameter("outT", [128, ECP], MMDT, isOutput=True)

    SILU = mybir.ActivationFunctionType.Silu

    with tile.TileContext(nc) as tc:
        with (
            tc.tile_pool(name="consts", bufs=1) as cpool,
            tc.tile_pool(name="xin", bufs=3) as xpool,
            tc.tile_pool(name="mid", bufs=2) as mpool,
            tc.tile_pool(name="outp", bufs=3) as opool,
            tc.tile_pool(name="ps", bufs=1, space="PSUM") as ps,
        ):
            # ---- load constants once (single DMA) -----------------------
            cb = cpool.tile([128, CB_COLS], MMDT, tag="cb", name="cb")
            nc.sync.dma_start(cb[:], cb_d[:])
            csb = {
                n: cb[r0:r0 + p, c0:c0 + w]
                for n, (r0, p, c0, w) in CB_LAYOUT.items()
            }

            for t in range(NT):
                e0 = t * TILE

                # ---- input DMAs ----------------------------------------
                xa = xpool.tile([128, TILE], MMDT, tag="xa")
                nc.sync.dma_start(xa[:], xa_d[:, e0:e0 + TILE])
                xb = xpool.tile([128, TILE], MMDT, tag="xb")
                nc.sync.dma_start(xb[:], xb_d[:, e0:e0 + TILE])
                sc = xpool.tile([68, TILE], MMDT, tag="sc")
                nc.sync.dma_start(sc[:], sc_d[:, e0:e0 + TILE])

                # ---- MLP layer 1 (PE waits only on sc) -----------------
                p1 = ps.tile([64, 512], F32, tag="PM", name="p1")
                nc.tensor.matmul(p1[:], csb["Wm1_8"], sc[0:64, :],
                                 start=True, stop=True)

                # ---- Sel matmuls: per-row-group y broadcast rows -------
                y_rows = sc[64:68, :]
                pBA = ps.tile([128, 512], F32, tag="SA", name="pBA")
                nc.tensor.matmul(pBA[:], csb["SelA"], y_rows,
                                 start=True, stop=True)
                pB1 = ps.tile([128, 512], F32, tag="SB", name="pB1")
                nc.tensor.matmul(pB1[:], csb["Sel1"], y_rows,
                                 start=True, stop=True)
                pB2 = ps.tile([64, 512], F32, tag="SC", name="pB2")
                nc.tensor.matmul(pB2[:], csb["Sel2"], y_rows,
                                 start=True, stop=True)
                BA = mpool.tile([128, 512], MMDT, tag="BA")
                nc.scalar.copy(BA[:], pBA[:])
                B1 = mpool.tile([128, 512], MMDT, tag="B1")
                nc.vector.tensor_copy(B1[:], pB1[:])
                B2 = mpool.tile([64, 512], MMDT, tag="B2")
                nc.scalar.copy(B2[:], pB2[:])

                # ---- MLP silu 1 (scalar) -------------------------------
                a1 = mpool.tile([64, 512], MMDT, tag="a1")
                nc.scalar.activation(a1[:], p1[:], SILU)

                # ---- G-stage (PE, independent of MLP) ------------------
                pGA = ps.tile([128, 512], F32, tag="GA", name="pGA")
                pGB = ps.tile([128, 512], F32, tag="GB", name="pGB")
                pGBx = ps.tile([64, 512], F32, tag="GC", name="pGBx")
                for si, (v, la, lb, lx) in enumerate(
                    ((xa, "LA_a", "LB_a", "LBx_a"),
                     (xb, "LA_b", "LB_b", "LBx_b"))
                ):
                    st = si == 0
                    sp = si == 1
                    nc.tensor.matmul(pGA[:], csb[la], v[:], start=st, stop=sp)
                    nc.tensor.matmul(pGB[:], csb[lb], v[:], start=st, stop=sp)
                    nc.tensor.matmul(pGBx[:], csb[lx], v[:], start=st, stop=sp)

                # ---- MLP layer 2 + silu + layer 3 ----------------------
                p2 = ps.tile([64, 512], F32, tag="PM", name="p2")
                nc.tensor.matmul(p2[:], csb["Wm2s"], a1[:],
                                 start=True, stop=True)
                a2h = mpool.tile([64, 512], MMDT, tag="a2")
                nc.scalar.activation(a2h[:], p2[:], SILU)

                pWTA = ps.tile([128, 512], F32, tag="WA", name="pWTA")
                nc.tensor.matmul(pWTA[:], csb["Wm3A"], a2h[:],
                                 start=True, stop=True)
                pWTB = ps.tile([128, 512], F32, tag="SA", name="pWTB")
                nc.tensor.matmul(pWTB[:], csb["Wm3B"], a2h[:],
                                 start=True, stop=True)
                pWTBx = ps.tile([64, 512], F32, tag="SB", name="pWTBx")
                nc.tensor.matmul(pWTBx[:], csb["Wm3Bx"], a2h[:],
                                 start=True, stop=True)

                # ---- elementwise (DVE): Ms = G * B * WT ----------------
                YA = mpool.tile([128, 512], MMDT, tag="YA")
                nc.vector.tensor_mul(YA[:], pGA[:], BA[:])
                Y1 = mpool.tile([128, 512], MMDT, tag="Y1")
                nc.vector.tensor_mul(Y1[:], pGB[:], B1[:])
                Y2 = mpool.tile([64, 512], MMDT, tag="Y2")
                nc.vector.tensor_mul(Y2[:], pGBx[:], B2[:])
                MsA = mpool.tile([128, 512], MMDT, tag="MsA")
                nc.vector.tensor_mul(MsA[:], YA[:], pWTA[:])
                MsB = mpool.tile([128, 512], MMDT, tag="MsB")
                nc.vector.tensor_mul(MsB[:], Y1[:], pWTB[:])
                MsBx = mpool.tile([64, 512], MMDT, tag="MsBx")
                nc.vector.tensor_mul(MsBx[:], Y2[:], pWTBx[:])

                # ---- F-stage -> feature-major output -------------------
                pOF = ps.tile([128, 512], F32, tag="SC", name="pOF")
                nc.tensor.matmul(pOF[:], csb["WfA"], MsA[:],
                                 start=True, stop=False)
                nc.tensor.matmul(pOF[:], csb["WfB"], MsB[:],
                                 start=False, stop=False)
                nc.tensor.matmul(pOF[:], csb["WfBx"], MsBx[:],
                                 start=False, stop=True)
                OF = opool.tile([128, 512], MMDT, tag="OF")
                nc.scalar.copy(OF[:], pOF[:])
                nc.sync.dma_start(out_d[:, e0:e0 + TILE], OF[:])

    nc.finalize()
    return nc


def _get_nc():
    if "nc" not in _CACHE:
        _CACHE["nc"] = build_nc()
    return _CACHE["nc"]


# ----------------------------------------------------------------------------
# host entry point


def _padT(aT):
    """[F, EC] -> [F, ECP] zero-padded bf16, contiguous."""
    out = np.zeros((aT.shape[0], ECP), dtype=NPBF16)
    out[:, :EC] = aT
    return out


def run(inputs, trace=False):
    inputs = {k: np.asarray(v) for k, v in inputs.items()}
    consts = build_consts(
        inputs["w0"], inputs["w1"], inputs["w2"], inputs["w3"],
        inputs["Wl0"], inputs["Wl1"],
        inputs["Wm1"], inputs["Wm2"], inputs["Wm3"],
        inputs["Wf0"], inputs["Wf1"],
    )
    cb = pack_consts(consts)
    nc = _get_nc()
    x1aT = inputs["x1a"].T          # [128, E] views
    x1bT = inputs["x1b"].T
    scx2T = np.concatenate([inputs["scalars"], inputs["x2"]], axis=1).T  # [68,E]
    in_maps = []
    for c in range(NCORES):
        s = slice(c * EC, (c + 1) * EC)
        m = {
            "x1aT": _padT(x1aT[:, s]),
            "x1bT": _padT(x1bT[:, s]),
            "scx2T": _padT(scx2T[:, s]),
            "CB": cb,
        }
        in_maps.append(m)
    res = run_bass_kernel_spmd(nc, in_maps, list(range(NCORES)), trace=trace)
    out = np.concatenate(
        [res.results[c]["outT"][:, :EC].astype(np.float32).T
         for c in range(NCORES)], axis=0)
    return np.ascontiguousarray(out), res


def kernel(**inputs) -> np.ndarray:
    out, _ = run(inputs, trace=False)
    return out
